# revision 1
# baseline (speedup 1.0000x reference)
"""DenseGIN (3-layer, dense adjacency) Trainium2 Bass kernel, 8-core SPMD.

Problem: x:(4,4096,2,32) f32, adj:(4,4096,4096) f32 binary, mask:(4,4096) bool.
Per layer l: agg = (adj+I) @ xf ; h = relu(agg@Wa+ba); h = BN(h); h = h@Wb+bb;
x = mask*h ; between layers an outer BN is applied at masked nodes.

Sharding: 8 cores = (batch b, node-half). Core (2b+h) owns output nodes
[h*2048,(h+1)*2048) of batch b.

Key design decisions:
- Host pre-transposes A = (adj[b] + I) and slices columns, so the device gets
  adjT[i, k] = A[k, i] in natural row-major layout.  The PE matmul
  out = lhsT.T @ rhs with lhsT = xf node-major tiles [128 nodes, KC chans] and
  rhs = adjT tiles [128 nodes_in, 512 nodes_out] then directly produces
  aggT[chan, node_out] with NO transposes anywhere on device.
- adjT is cast to bf16 on host (exact: entries are 0/1) and kept RESIDENT in
  SBUF (16 MiB) across all three layers -> adjacency is read from HBM once.
- xf is split into hi+lo bf16 parts (pseudo-fp32): agg accumulates
  A@hi + A@lo in fp32 PSUM; quantization error ~2^-17 relative.
- All BN affines are folded on the host: Wb'' = diag(s1_bn)*Wb*diag(s2_outer),
  d = (bb + c1@Wb)*s2 + c2, so the device epilogue per node tile is just
  (psum + D) * mask_column.
- Between layers, node halves are exchanged with a 2-core AllGather of the
  packed [2048, 256] bf16 (hi|lo) activation tensor.
"""

import sys

if "/opt/trn_rl_repo" not in sys.path:  # PYTHONPATH normally provides it
    sys.path.insert(0, "/opt/trn_rl_repo")

import contextlib
import ctypes
import types

import numpy as np
import ml_dtypes

import concourse.bass as bass
import concourse.tile as tile
from concourse import mybir
from concourse.vector_clock import ScopedClock
import concourse.bass_utils as bass_utils
from concourse.bass_utils import run_bass_kernel_spmd

# ---------------------------------------------------------------------------
# Workaround: the walrus build in this container rejects instructions with
# more than one sem wait ("Too many sync wait commands").  Tile's final drain
# attaches one wait per live semaphore; split them across chained SP drains.
_MAX_WAITS_PER_INST = 1


def _patched_drain_and_barrier(self, tick_clock, wait_clock):
    nc = self.nc
    drain_inst = nc.sync.drain()
    wait_clock.add_sem_waits(drain_inst.ins, ScopedClock({None: tick_clock.global_clock}))
    si = drain_inst.ins.sync_info
    waits = list(si.on_wait or [])
    if len(waits) > _MAX_WAITS_PER_INST:
        si.on_wait = waits[:_MAX_WAITS_PER_INST]
        rest = waits[_MAX_WAITS_PER_INST:]
        for i in range(0, len(rest), _MAX_WAITS_PER_INST):
            extra = nc.sync.drain()
            extra.ins.sync_info = mybir.SyncInfo(
                on_wait=rest[i : i + _MAX_WAITS_PER_INST], on_update=[]
            )
    nc.all_engine_barrier()
    assert self.sems is not None
    popped = nc._tile_sem_poison_stack.pop()
    assert popped is self._sem_poison
    nc.clear_and_free_semaphores(list(self.sems.allocated().values()))
    nc.all_engine_barrier()


tile.TileContext._drain_and_barrier = _patched_drain_and_barrier


def _legalize_sync_waits(nc, max_waits=_MAX_WAITS_PER_INST):
    """Split instructions carrying more than ``max_waits`` sem waits.

    Engine sequencers process their instruction stream in order and execute
    sem waits before dispatch, so hoisting excess waits onto NoOps placed
    just before the instruction (same engine) is semantics-preserving.
    """
    n_split = 0
    for fn in nc.m.functions:
        for blk in fn.blocks:
            insts = blk.instructions
            i = 0
            while i < len(insts):
                inst = insts[i]
                si = inst.sync_info
                waits = list(si.on_wait) if si and si.on_wait else []
                if len(waits) > max_waits:
                    extra, keep = waits[:-max_waits], waits[-max_waits:]
                    si.on_wait = keep
                    pos = i
                    for j in range(0, len(extra), max_waits):
                        nop = mybir.InstNoOp(name=f"I-lsw{n_split}-{j}", ins=[], outs=[])
                        nop.engine = inst.engine
                        nop.sync_info = mybir.SyncInfo(
                            on_wait=extra[j : j + max_waits], on_update=[]
                        )
                        insts.insert(pos, nop)
                        pos += 1
                        i += 1
                    n_split += 1
                i += 1
    return n_split


# ---------------------------------------------------------------------------
# NTFF profiling hook (antenv.axon_hooks is absent in this image).  Only used
# when run() is called with trace=True; registering it is harmless otherwise.
def _ntff_profile_via_ctypes(so_path):
    try:
        lib = ctypes.CDLL(so_path)
    except OSError:
        return None
    if not hasattr(lib, "axon_start_nrt_profile"):
        return None
    lib.axon_start_nrt_profile.argtypes = [ctypes.POINTER(ctypes.c_int64), ctypes.c_size_t]
    lib.axon_start_nrt_profile.restype = ctypes.c_int64
    lib.axon_stop_nrt_profile.argtypes = [ctypes.c_char_p]
    lib.axon_stop_nrt_profile.restype = ctypes.c_int64

    @contextlib.contextmanager
    def _hook(output_dir, device_ids):
        import jax

        jax.devices()
        if device_ids:
            ids = (ctypes.c_int64 * len(device_ids))(*device_ids)
            rc = lib.axon_start_nrt_profile(ids, len(device_ids))
        else:
            rc = lib.axon_start_nrt_profile(None, 0)
        if rc != 0:
            raise RuntimeError(f"axon_start_nrt_profile rc={rc}")
        try:
            yield
        finally:
            n = lib.axon_stop_nrt_profile(str(output_dir).encode())
            print(f"ntff profile: {n} file(s) written to {output_dir}", file=sys.stderr)

    return _hook


if "antenv.axon_hooks" not in sys.modules:
    _hooks_mod = types.ModuleType("antenv.axon_hooks")
    _hook_inst = _ntff_profile_via_ctypes("/opt/axon/libaxon_pjrt.so")
    _hooks_mod.get_axon_ntff_profile_hook = lambda: _hook_inst
    sys.modules["antenv.axon_hooks"] = _hooks_mod
bass_utils.upload_artifacts = lambda tmpdir: f"local:{tmpdir}"

# ---------------------------------------------------------------------------
B, N, K, C_IN, H, C_OUT = 4, 4096, 2, 32, 64, 32
BN_EPS = 1e-5
N_CORES = 8
HALF = N // 2          # 2048 output nodes per core
NT = N // 128          # 32 node tiles (contraction side)
KC_IN = [K * C_IN, K * H, K * H]     # flat input channels per layer: 64,128,128
KC_OUT = [K * H, K * H, K * C_OUT]   # flat output channels per layer: 128,128,64
CO = [H, H, C_OUT]                   # per-k output channels: 64,64,32
CI = [C_IN, H, H]                    # per-k input channels: 32,64,64

BF16 = ml_dtypes.bfloat16

_PROGRAM_CACHE = {}


def _build_program(n_layers=3, use_cc=True):
    """Build the SPMD Bass/Tile program (identical on all 8 cores)."""
    nc = bass.Bass("TRN2", target_bir_lowering=False, debug=False, num_devices=N_CORES)
    dt = mybir.dt

    adjT_d = nc.dram_tensor("adjT", [N, HALF], dt.bfloat16, kind="ExternalInput").ap()
    xh0_d = nc.dram_tensor("xh0", [N, KC_IN[0]], dt.bfloat16, kind="ExternalInput").ap()
    xl0_d = nc.dram_tensor("xl0", [N, KC_IN[0]], dt.bfloat16, kind="ExternalInput").ap()
    mask_d = nc.dram_tensor("mask_cols", [128, 16], dt.float32, kind="ExternalInput").ap()
    # Wa/Wb are stored block-diagonally over the K=2 slice structure so each
    # MLP stage is a single full-partition matmul with base_partition 0
    # (partition-offset matmul operands crash at runtime on this stack).
    wa_d = [
        nc.dram_tensor(f"wa{l}", [KC_IN[l], 2 * H], dt.float32, kind="ExternalInput").ap()
        for l in range(3)
    ]
    wb_d = [
        nc.dram_tensor(f"wb{l}", [2 * H, KC_OUT[l]], dt.float32, kind="ExternalInput").ap()
        for l in range(3)
    ]
    ba_d = [
        nc.dram_tensor(f"ba{l}", [128, 1], dt.float32, kind="ExternalInput").ap()
        for l in range(3)
    ]
    dd_d = [
        nc.dram_tensor(f"d{l}", [128, KC_OUT[l]], dt.float32, kind="ExternalInput").ap()
        for l in range(3)
    ]
    out_d = nc.dram_tensor(
        "out", [HALF, KC_OUT[n_layers - 1]], dt.float32, kind="ExternalOutput"
    ).ap()

    with tile.TileContext(nc) as tc:
        with (
            tc.tile_pool(name="const", bufs=1) as cpool,
            tc.tile_pool(name="xf", bufs=2) as xpool,
            tc.tile_pool(name="work", bufs=3) as wpool,
            tc.tile_pool(name="ps_agg", bufs=2, space="PSUM") as ps_agg,
            tc.tile_pool(name="ps_mlp", bufs=2, space="PSUM") as ps_mlp,
            tc.tile_pool(name="dram", bufs=2, space="DRAM") as dpool,
        ):
            # --- HAM warmup: dummy matmuls keep the PE clock at 8/8 while the
            # initial adjacency DMA streams in (operand contents irrelevant) ---
            wu_lhs = cpool.tile([128, 128], dt.bfloat16, tag="wu_lhs")
            wu_rhs = cpool.tile([128, 512], dt.bfloat16, tag="wu_rhs")
            nc.gpsimd.memset(wu_lhs[:], 0.0)
            nc.gpsimd.memset(wu_rhs[:], 0.0)
            wu_ps = ps_mlp.tile([128, 512], dt.float32, tag="h1")
            for _ in range(28):
                nc.tensor.matmul(wu_ps[:], wu_lhs[:], wu_rhs[:], start=True, stop=True)

            # --- layer-0 activations first (small, unblocks first matmuls) ---
            kc0 = KC_IN[0]
            xh_sb = [
                xpool.tile([128, kc0], dt.bfloat16, tag=f"xh{i}", name=f"xh0_{i}")
                for i in range(NT)
            ]
            xl_sb = [
                xpool.tile([128, kc0], dt.bfloat16, tag=f"xl{i}", name=f"xl0_{i}")
                for i in range(NT)
            ]
            for i in range(NT):
                nc.gpsimd.dma_start(xh_sb[i][:], xh0_d[i * 128 : (i + 1) * 128, :])
                nc.gpsimd.dma_start(xl_sb[i][:], xl0_d[i * 128 : (i + 1) * 128, :])

            # --- resident adjacency: 32 x [128, 2048] bf16 = 16 MiB total.
            # Separate tiles so each accumulation matmul only depends on its
            # own slice's DMA (whole-tile deps would serialize layer 0). ---
            adjT_sb = [
                cpool.tile([128, HALF], dt.bfloat16, tag=f"adjT{i}", name=f"adjT_{i}")
                for i in range(NT)
            ]
            for i in range(NT):
                nc.sync.dma_start(adjT_sb[i][:], adjT_d[i * 128 : (i + 1) * 128, :])

            # --- constants ---
            mask_sb = cpool.tile([128, 16], dt.float32, tag="mask")
            nc.gpsimd.dma_start(mask_sb[:], mask_d[:])
            wa_sb, wb_sb, ba_sb, dd_sb = [], [], [], []
            for l in range(3):
                wa = cpool.tile([KC_IN[l], 2 * H], dt.float32, tag=f"wa{l}")
                nc.gpsimd.dma_start(wa[:], wa_d[l][:])
                wa_sb.append(wa)
                wb = cpool.tile([2 * H, KC_OUT[l]], dt.float32, tag=f"wb{l}")
                nc.gpsimd.dma_start(wb[:], wb_d[l][:])
                wb_sb.append(wb)
                ba = cpool.tile([128, 1], dt.float32, tag=f"ba{l}")
                nc.gpsimd.dma_start(ba[:], ba_d[l][:])
                ba_sb.append(ba)
                dd = cpool.tile([128, KC_OUT[l]], dt.float32, tag=f"d{l}")
                nc.gpsimd.dma_start(dd[:], dd_d[l][:])
                dd_sb.append(dd)

            for l in range(n_layers):
                kci, kco = KC_IN[l], KC_OUT[l]
                last = l == n_layers - 1
                if not last:
                    # one AllGather per 512-node chunk, launched as soon as the
                    # chunk's epilogue finishes, so collective latency hides
                    # behind the remaining chunks' compute.  Output rows are
                    # global-node-indexed: [0:512]=pair-rank0's chunk,
                    # [512:1024]=rank1's chunk -> identical addressing on all
                    # cores (the NEFF is shared).
                    ag_in = [
                        dpool.tile([512, 2 * kco], dt.bfloat16, tag=f"ag_in{c}", name=f"ag_in_l{l}_{c}")
                        for c in range(4)
                    ]
                    ag_out = [
                        dpool.tile([1024, 2 * kco], dt.bfloat16, tag=f"ag_out{c}", name=f"ag_out_l{l}_{c}")
                        for c in range(4)
                    ]

                # contraction order: layer 0 streams tiles in DMA order; later
                # layers consume in chunk-arrival order (AG_c completes ~in
                # launch order), both halves of each chunk together.
                if l == 0:
                    i_order = list(range(NT))
                else:
                    i_order = []
                    for c in range(4):
                        i_order += [c * 4 + t for t in range(4)]
                        i_order += [16 + c * 4 + t for t in range(4)]

                if not last:
                    kcn = KC_IN[l + 1]
                    assert kcn == kco
                    xh_next = [None] * NT
                    xl_next = [None] * NT

                for kc in range(4):
                    # ---- aggregation for this 512-node output chunk:
                    # aggT[chan, node] = sum_i xf_i.T @ adjT_i ----
                    agg_ps = ps_agg.tile(
                        [kci, 512], dt.float32, tag="agg", name=f"agg_l{l}_{kc}"
                    )
                    for ii, i in enumerate(i_order):
                        rhs = adjT_sb[i][:, kc * 512 : (kc + 1) * 512]
                        nc.tensor.matmul(
                            agg_ps[:], xh_sb[i][:], rhs, start=(ii == 0), stop=False
                        )
                        nc.tensor.matmul(
                            agg_ps[:], xl_sb[i][:], rhs, start=False, stop=(ii == NT - 1)
                        )
                    agg_sb = wpool.tile([kci, 512], dt.float32, tag="agg_sb")
                    nc.scalar.copy(agg_sb[:], agg_ps[:])

                    # ---- MLP matmul 1 (block-diagonal Wa) + bias + relu ----
                    h1_ps = ps_mlp.tile([128, 512], dt.float32, tag="h1")
                    nc.tensor.matmul(
                        h1_ps[:], wa_sb[l][:], agg_sb[:], start=True, stop=True
                    )
                    h1_sb = wpool.tile([128, 512], dt.float32, tag="h1_sb")
                    nc.scalar.activation(
                        h1_sb[:],
                        h1_ps[:],
                        mybir.ActivationFunctionType.Relu,
                        bias=ba_sb[l][:, 0:1],
                    )

                    # ---- MLP matmul 2 (node-major) + epilogue per 128-node tile ----
                    for t in range(4):
                        nsl = slice(t * 128, (t + 1) * 128)
                        xn_ps = ps_mlp.tile([128, kco], dt.float32, tag="xn")
                        nc.tensor.matmul(
                            xn_ps[:], h1_sb[:, nsl], wb_sb[l][:], start=True, stop=True
                        )
                        # (psum + D) * mask
                        xn_sb = wpool.tile([128, kco], dt.float32, tag="xn_sb")
                        nc.vector.tensor_add(xn_sb[:], xn_ps[:], dd_sb[l][:])
                        xm_sb = wpool.tile([128, kco], dt.float32, tag="xm_sb")
                        mcol = mask_sb[:, kc * 4 + t : kc * 4 + t + 1]
                        nc.scalar.activation(
                            xm_sb[:],
                            xn_sb[:],
                            mybir.ActivationFunctionType.Copy,
                            scale=mcol,
                        )
                        rows = slice((kc * 4 + t) * 128, (kc * 4 + t + 1) * 128)
                        if not last:
                            hi_sb = wpool.tile([128, kco], dt.bfloat16, tag="hi")
                            nc.vector.tensor_copy(hi_sb[:], xm_sb[:])
                            hif_sb = wpool.tile([128, kco], dt.float32, tag="hif")
                            nc.vector.tensor_copy(hif_sb[:], hi_sb[:])
                            lo_sb = wpool.tile([128, kco], dt.bfloat16, tag="lo")
                            nc.vector.tensor_sub(lo_sb[:], xm_sb[:], hif_sb[:])
                            trows = slice(t * 128, (t + 1) * 128)
                            nc.sync.dma_start(ag_in[kc][trows, 0:kco], hi_sb[:])
                            nc.sync.dma_start(ag_in[kc][trows, kco : 2 * kco], lo_sb[:])
                        else:
                            nc.sync.dma_start(out_d[rows, :], xm_sb[:])

                    if not last:
                        if use_cc:
                            nc.gpsimd.collective_compute(
                                "AllGather",
                                mybir.AluOpType.bypass,
                                replica_groups=[[0, 1], [2, 3], [4, 5], [6, 7]],
                                ins=[ag_in[kc].opt()],
                                outs=[ag_out[kc].opt()],
                            )
                        else:
                            nc.sync.dma_start(ag_out[kc][0:512, :], ag_in[kc][:, :])
                        # next layer's lhsT tiles for this chunk (both halves)
                        for t in range(4):
                            srows = slice(t * 128, (t + 1) * 128)
                            prows = slice(512 + t * 128, 512 + (t + 1) * 128)
                            j0, j1 = kc * 4 + t, 16 + kc * 4 + t
                            xh_next[j0] = xpool.tile([128, kcn], dt.bfloat16, tag=f"xh{j0}", name=f"xh_l{l}_{j0}")
                            xl_next[j0] = xpool.tile([128, kcn], dt.bfloat16, tag=f"xl{j0}", name=f"xl_l{l}_{j0}")
                            xh_next[j1] = xpool.tile([128, kcn], dt.bfloat16, tag=f"xh{j1}", name=f"xh_l{l}_{j1}")
                            xl_next[j1] = xpool.tile([128, kcn], dt.bfloat16, tag=f"xl{j1}", name=f"xl_l{l}_{j1}")
                            nc.gpsimd.dma_start(xh_next[j0][:], ag_out[kc][srows, 0:kcn])
                            nc.gpsimd.dma_start(xl_next[j0][:], ag_out[kc][srows, kcn : 2 * kcn])
                            nc.gpsimd.dma_start(xh_next[j1][:], ag_out[kc][prows, 0:kcn])
                            nc.gpsimd.dma_start(xl_next[j1][:], ag_out[kc][prows, kcn : 2 * kcn])

                if not last:
                    xh_sb, xl_sb = xh_next, xl_next

    n_split = _legalize_sync_waits(nc)
    print(f"kernel: legalized {n_split} multi-wait instructions", file=sys.stderr)
    return nc


def get_program():
    if "nc" not in _PROGRAM_CACHE:
        _PROGRAM_CACHE["nc"] = _build_program()
    return _PROGRAM_CACHE["nc"]


def prepare_in_maps(inputs):
    """Host-side prep: fold BN into weights, transpose+slice adjacency, split x."""
    f32 = np.float32
    x = np.asarray(inputs["x"], f32)
    adj = np.asarray(inputs["adj"], f32)
    mask = np.asarray(inputs["mask"]).astype(bool)

    # folded per-layer constants (shared by all cores)
    const = {}
    for l in range(3):
        Wa = np.asarray(inputs[f"Wa{l}"], f32)
        ba = np.asarray(inputs[f"ba{l}"], f32)
        Wb = np.asarray(inputs[f"Wb{l}"], f32)
        bb = np.asarray(inputs[f"bb{l}"], f32)
        s1 = np.asarray(inputs[f"bng{l}"], f32) / np.sqrt(
            np.asarray(inputs[f"bnv{l}"], f32) + BN_EPS
        )
        c1 = np.asarray(inputs[f"bnb{l}"], f32) - np.asarray(inputs[f"bnm{l}"], f32) * s1
        Wb1 = s1[:, None] * Wb
        bb1 = bb + c1 @ Wb
        if l < 2:
            s2 = np.asarray(inputs[f"og{l}"], f32) / np.sqrt(
                np.asarray(inputs[f"ov{l}"], f32) + BN_EPS
            )
            c2 = np.asarray(inputs[f"ob{l}"], f32) - np.asarray(inputs[f"om{l}"], f32) * s2
            Wb2 = (Wb1 * s2[None, :]).astype(f32)
            d = (bb1 * s2 + c2).astype(f32)
        else:
            Wb2 = Wb1.astype(f32)
            d = bb1.astype(f32)
        dtile = np.broadcast_to(
            np.concatenate([d, d])[None, :], (128, 2 * d.shape[0])
        ).copy()
        ci, co = Wa.shape[0], Wb2.shape[1]
        waBD = np.zeros((2 * ci, 2 * H), f32)
        wbBD = np.zeros((2 * H, 2 * co), f32)
        for k in range(2):
            waBD[k * ci : (k + 1) * ci, k * H : (k + 1) * H] = Wa
            wbBD[k * H : (k + 1) * H, k * co : (k + 1) * co] = Wb2
        const[f"wa{l}"] = waBD
        const[f"wb{l}"] = wbBD
        const[f"ba{l}"] = np.concatenate([ba, ba]).reshape(128, 1).astype(f32)
        const[f"d{l}"] = dtile.astype(f32)

    in_maps = []
    for core in range(N_CORES):
        b, half = divmod(core, 2)
        r0 = half * HALF
        # adjT[i, j] = adj[b][r0+j, i] + I  -> natural layout for rhs tiles
        adjT = np.ascontiguousarray(adj[b][r0 : r0 + HALF, :].T)
        adjT[np.arange(HALF) + r0, np.arange(HALF)] += 1.0
        xb = x[b].reshape(N, KC_IN[0])
        xh = xb.astype(BF16)
        xl = (xb - xh.astype(f32)).astype(BF16)
        mhalf = mask[b][r0 : r0 + HALF].astype(f32)
        m = dict(const)
        m["adjT"] = adjT.astype(BF16)
        m["xh0"] = xh
        m["xl0"] = xl
        m["mask_cols"] = np.ascontiguousarray(mhalf.reshape(16, 128).T)
        in_maps.append(m)
    return in_maps


def run(in_maps, trace=False, **kw):
    nc = get_program()
    return run_bass_kernel_spmd(nc, in_maps, list(range(N_CORES)), trace=trace, **kw)


def kernel(**inputs) -> np.ndarray:
    in_maps = prepare_in_maps(inputs)
    res = run(in_maps)
    out = np.zeros((B, N, K, C_OUT), np.float32)
    for core in range(N_CORES):
        b, half = divmod(core, 2)
        r0 = half * HALF
        out[b, r0 : r0 + HALF] = res.results[core]["out"].reshape(HALF, K, C_OUT)
    return out



# revision 5
# speedup vs baseline: 1.7575x; 1.7575x over previous
"""DenseGIN (3-layer, dense adjacency) Trainium2 Bass kernel, 8-core SPMD.

Problem: x:(4,4096,2,32) f32, adj:(4,4096,4096) f32 binary, mask:(4,4096) bool.
Per layer l: agg = (adj+I) @ xf ; h = relu(agg@Wa+ba); h = BN(h); h = h@Wb+bb;
x = mask*h ; between layers an outer BN is applied at masked nodes.

Sharding: 8 cores = (batch b, node-half). Core (2b+h) owns output nodes
[h*2048,(h+1)*2048) of batch b.

v2 design (rel-err gate is 2e-2; measured margins from exact host sim):
- Adjacency is cast to fp8 e4m3 on host (exact: entries are 0/1) and kept
  RESIDENT in SBUF (8 MiB) in a pair-interleaved layout adjp[t] =
  [128, 2, 2048] covering input node tiles (2t, 2t+1).
- Layers 0 and 1 aggregate with fp8 DoubleRow matmuls (2 k-tiles per pass,
  0.5 cycles/row): lhsT = x pair tiles [128, 2, <=64ch] fp8, rhs = adjacency
  pair slices [128, 2, 512]. Host sim: end-to-end rel err 8.4e-3.
- Layer 2 aggregates with bf16 stationary x against the SAME fp8 adjacency
  (mixed-dtype matmul, 1 cycle/row) - layer-2 x in fp8 would breach the gate.
- Tile-major accumulation: each pair streams into all live chunk PSUMs, so
  LDWEIGHTS is amortized 2-4x. Layer 1 runs as two half-layers (2 chunks
  each) to stay within 8 PSUM banks.
- MLP runs in bf16 (wa/wb bf16, agg and h1 cast on the psum->sbuf copy).
  All BN affines folded on host into Wb and a per-channel bias D.
- Per-chunk epilogue: (psum + D4) on DVE, then mask-scale + dtype cast on
  ACT into a [128, 4, co] chunk tile (fp8 for L0->L1, bf16 for L1->L2).
- Chunk tiles bounce p-major [128, 4*co] through DRAM, 2-core AllGather,
  read back [rank0|rank1] halves; adjacency rows are in global node order so
  the NEFF is rank-independent (baseline trick).
- Output is written p-major [128, 16*64] and unshuffled on host.
"""

import sys

if "/opt/trn_rl_repo" not in sys.path:  # PYTHONPATH normally provides it
    sys.path.insert(0, "/opt/trn_rl_repo")

import contextlib
import ctypes
import types

import numpy as np
import ml_dtypes

import concourse.bass as bass
import concourse.tile as tile
from concourse import mybir
from concourse.vector_clock import ScopedClock
import concourse.bass_utils as bass_utils
from concourse.bass_utils import run_bass_kernel_spmd

# ---------------------------------------------------------------------------
# Workaround: the walrus build in this container rejects instructions with
# more than one sem wait ("Too many sync wait commands").  Tile's final drain
# attaches one wait per live semaphore; split them across chained SP drains.
_MAX_WAITS_PER_INST = 1


def _patched_drain_and_barrier(self, tick_clock, wait_clock):
    nc = self.nc
    drain_inst = nc.sync.drain()
    wait_clock.add_sem_waits(drain_inst.ins, ScopedClock({None: tick_clock.global_clock}))
    si = drain_inst.ins.sync_info
    waits = list(si.on_wait or [])
    if len(waits) > _MAX_WAITS_PER_INST:
        si.on_wait = waits[:_MAX_WAITS_PER_INST]
        rest = waits[_MAX_WAITS_PER_INST:]
        for i in range(0, len(rest), _MAX_WAITS_PER_INST):
            extra = nc.sync.drain()
            extra.ins.sync_info = mybir.SyncInfo(
                on_wait=rest[i : i + _MAX_WAITS_PER_INST], on_update=[]
            )
    nc.all_engine_barrier()
    assert self.sems is not None
    popped = nc._tile_sem_poison_stack.pop()
    assert popped is self._sem_poison
    nc.clear_and_free_semaphores(list(self.sems.allocated().values()))
    nc.all_engine_barrier()


tile.TileContext._drain_and_barrier = _patched_drain_and_barrier


def _legalize_sync_waits(nc, max_waits=_MAX_WAITS_PER_INST):
    """Split instructions carrying more than ``max_waits`` sem waits.

    Engine sequencers process their instruction stream in order and execute
    sem waits before dispatch, so hoisting excess waits onto NoOps placed
    just before the instruction (same engine) is semantics-preserving.
    """
    n_split = 0
    for fn in nc.m.functions:
        for blk in fn.blocks:
            insts = blk.instructions
            i = 0
            while i < len(insts):
                inst = insts[i]
                si = inst.sync_info
                waits = list(si.on_wait) if si and si.on_wait else []
                if len(waits) > max_waits:
                    extra, keep = waits[:-max_waits], waits[-max_waits:]
                    si.on_wait = keep
                    pos = i
                    for j in range(0, len(extra), max_waits):
                        nop = mybir.InstNoOp(name=f"I-lsw{n_split}-{j}", ins=[], outs=[])
                        nop.engine = inst.engine
                        nop.sync_info = mybir.SyncInfo(
                            on_wait=extra[j : j + max_waits], on_update=[]
                        )
                        insts.insert(pos, nop)
                        pos += 1
                        i += 1
                    n_split += 1
                i += 1
    return n_split


# ---------------------------------------------------------------------------
# NTFF profiling hook (antenv.axon_hooks is absent in this image).  Only used
# when run() is called with trace=True; registering it is harmless otherwise.
def _ntff_profile_via_ctypes(so_path):
    try:
        lib = ctypes.CDLL(so_path)
    except OSError:
        return None
    if not hasattr(lib, "axon_start_nrt_profile"):
        return None
    lib.axon_start_nrt_profile.argtypes = [ctypes.POINTER(ctypes.c_int64), ctypes.c_size_t]
    lib.axon_start_nrt_profile.restype = ctypes.c_int64
    lib.axon_stop_nrt_profile.argtypes = [ctypes.c_char_p]
    lib.axon_stop_nrt_profile.restype = ctypes.c_int64

    @contextlib.contextmanager
    def _hook(output_dir, device_ids):
        import jax

        jax.devices()
        if device_ids:
            ids = (ctypes.c_int64 * len(device_ids))(*device_ids)
            rc = lib.axon_start_nrt_profile(ids, len(device_ids))
        else:
            rc = lib.axon_start_nrt_profile(None, 0)
        if rc != 0:
            raise RuntimeError(f"axon_start_nrt_profile rc={rc}")
        try:
            yield
        finally:
            n = lib.axon_stop_nrt_profile(str(output_dir).encode())
            print(f"ntff profile: {n} file(s) written to {output_dir}", file=sys.stderr)

    return _hook


if "antenv.axon_hooks" not in sys.modules:
    _hooks_mod = types.ModuleType("antenv.axon_hooks")
    _hook_inst = _ntff_profile_via_ctypes("/opt/axon/libaxon_pjrt.so")
    _hooks_mod.get_axon_ntff_profile_hook = lambda: _hook_inst
    sys.modules["antenv.axon_hooks"] = _hooks_mod
bass_utils.upload_artifacts = lambda tmpdir: f"local:{tmpdir}"

# ---------------------------------------------------------------------------
B, N, K, C_IN, H, C_OUT = 4, 4096, 2, 32, 64, 32
BN_EPS = 1e-5
N_CORES = 8
HALF = N // 2          # 2048 output nodes per core
NPAIR = 16             # 16 pairs of 128-node input tiles
KC_IN = [K * C_IN, K * H, K * H]     # flat input channels per layer: 64,128,128
KC_OUT = [K * H, K * H, K * C_OUT]   # flat output channels per layer: 128,128,64
CO2 = [2 * H, 2 * H, 2 * C_OUT]      # = KC_OUT

BF16 = ml_dtypes.bfloat16
FP8 = ml_dtypes.float8_e4m3

_PROGRAM_CACHE = {}


def _build_program():
    """Build the SPMD Bass/Tile program (identical on all 8 cores)."""
    nc = bass.Bass("TRN2", target_bir_lowering=False, debug=False, num_devices=N_CORES)
    dt = mybir.dt
    DR = mybir.MatmulPerfMode.DoubleRow

    adjp_d = nc.dram_tensor("adjp", [NPAIR * 128, 2 * HALF], dt.float8e4, kind="ExternalInput").ap()
    x0q_d = nc.dram_tensor("x0q", [128, 32 * KC_IN[0]], dt.float8e4, kind="ExternalInput").ap()
    mask_d = nc.dram_tensor("mask_cols", [128, 16], dt.float32, kind="ExternalInput").ap()
    wa0_d = nc.dram_tensor("wa0", [64, 128], dt.bfloat16, kind="ExternalInput").ap()
    wa1a_d = nc.dram_tensor("wa1a", [64, 128], dt.bfloat16, kind="ExternalInput").ap()
    wa1b_d = nc.dram_tensor("wa1b", [64, 128], dt.bfloat16, kind="ExternalInput").ap()
    wa2_d = nc.dram_tensor("wa2", [128, 128], dt.bfloat16, kind="ExternalInput").ap()
    wb_d = [
        nc.dram_tensor(f"wb{l}", [128, KC_OUT[l]], dt.bfloat16, kind="ExternalInput").ap()
        for l in range(3)
    ]
    ba_d = [
        nc.dram_tensor(f"ba{l}", [128, 1], dt.float32, kind="ExternalInput").ap()
        for l in range(3)
    ]
    dd4_d = [
        nc.dram_tensor(f"dd4_{l}", [128, 4 * KC_OUT[l]], dt.float32, kind="ExternalInput").ap()
        for l in range(3)
    ]
    out_d = nc.dram_tensor("out", [128, 16 * KC_OUT[2]], dt.float32, kind="ExternalOutput").ap()

    with tile.TileContext(nc) as tc:
        with (
            tc.tile_pool(name="const", bufs=1) as cpool,
            tc.tile_pool(name="xio", bufs=1) as xpool,
            tc.tile_pool(name="work", bufs=3) as wpool,
            tc.tile_pool(name="ps_agg", bufs=1, space="PSUM") as ps_agg,
            tc.tile_pool(name="ps_mlp", bufs=2, space="PSUM") as ps_mlp,
            tc.tile_pool(name="dram", bufs=1, space="DRAM") as dpool,
        ):
            # ---- input DMAs, spread across engine queues ----
            x0_sb = xpool.tile([128, 32, KC_IN[0]], dt.float8e4, tag="x0")
            nc.gpsimd.dma_start(x0_sb[:, :, :], x0q_d[:, :])
            mask_sb = cpool.tile([128, 16], dt.float32, tag="mask")
            nc.gpsimd.dma_start(mask_sb[:], mask_d[:])

            adjp_sb = [
                cpool.tile([128, 2, HALF], dt.float8e4, tag=f"adjp{t}", name=f"adjp_{t}")
                for t in range(NPAIR)
            ]
            for t in range(NPAIR):
                src = adjp_d[t * 128 : (t + 1) * 128, :]
                if t % 2 == 0:
                    nc.sync.dma_start(adjp_sb[t][:, :, :], src)
                else:
                    nc.scalar.dma_start(adjp_sb[t][:, :, :], src)

            wa0_sb = cpool.tile([64, 128], dt.bfloat16, tag="wa0")
            nc.gpsimd.dma_start(wa0_sb[:], wa0_d[:])
            wa1a_sb = cpool.tile([64, 128], dt.bfloat16, tag="wa1a")
            nc.gpsimd.dma_start(wa1a_sb[:], wa1a_d[:])
            wa1b_sb = cpool.tile([64, 128], dt.bfloat16, tag="wa1b")
            nc.gpsimd.dma_start(wa1b_sb[:], wa1b_d[:])
            wa2_sb = cpool.tile([128, 128], dt.bfloat16, tag="wa2")
            nc.gpsimd.dma_start(wa2_sb[:], wa2_d[:])
            wb_sb, ba_sb, dd4_sb = [], [], []
            for l in range(3):
                wb = cpool.tile([128, KC_OUT[l]], dt.bfloat16, tag=f"wb{l}")
                nc.gpsimd.dma_start(wb[:], wb_d[l][:])
                wb_sb.append(wb)
                ba = cpool.tile([128, 1], dt.float32, tag=f"ba{l}")
                nc.gpsimd.dma_start(ba[:], ba_d[l][:])
                ba_sb.append(ba)
                dd4 = cpool.tile([128, 4 * KC_OUT[l]], dt.float32, tag=f"dd4_{l}")
                nc.gpsimd.dma_start(dd4[:], dd4_d[l][:])
                dd4_sb.append(dd4)

            # ---- HAM warmup: dummy matmuls ramp the PE clock to 8/8 while
            # the first input DMAs stream in (operand contents irrelevant) ----
            wu_lhs = cpool.tile([128, 128], dt.bfloat16, tag="wu_lhs")
            wu_rhs = cpool.tile([128, 512], dt.bfloat16, tag="wu_rhs")
            nc.gpsimd.memset(wu_lhs[:], 0.0)
            nc.gpsimd.memset(wu_rhs[:], 0.0)
            wu_ps = ps_mlp.tile([128, 512], dt.float32, tag="h1")
            for _ in range(10):
                nc.tensor.matmul(wu_ps[:], wu_lhs[:], wu_rhs[:], start=True, stop=True)

            # AllGather bounce buffers + gathered-x tiles per boundary.
            # xr[rk][c] rows are rank rk's chunk c = global input node tiles
            # [rk*16 + c*4, +4) in [p, subtile, chan] layout.
            def make_ag(l, kco, dtt):
                agin = [
                    dpool.tile([128, 4 * kco], dtt, tag=f"agin{l}_{c}", name=f"agin{l}_{c}")
                    for c in range(4)
                ]
                agout = [
                    dpool.tile([256, 4 * kco], dtt, tag=f"agout{l}_{c}", name=f"agout{l}_{c}")
                    for c in range(4)
                ]
                xr = [
                    [
                        xpool.tile([128, 4, kco], dtt, tag=f"xr{l}_{rk}_{c}", name=f"xr{l}_{rk}_{c}")
                        for c in range(4)
                    ]
                    for rk in range(2)
                ]
                return agin, agout, xr

            agin1, agout1, xr1 = make_ag(0, KC_OUT[0], dt.float8e4)
            agin2, agout2, xr2 = make_ag(1, KC_OUT[1], dt.bfloat16)

            def mlp_epilogue(l, c, agg_mms, agg_list, wa_list, xc_tile, last):
                """MLP + epilogue for output chunk c of layer l.

                agg_list: list of (agg_psum_ap, contraction_partitions) whose
                bf16 copies accumulate into h1 with the matching wa_list lhsT.
                """
                kco = KC_OUT[l]
                h1_ps = ps_mlp.tile([128, 512], dt.float32, tag="h1", name=f"h1_{l}_{c}")
                asbs = []
                for gi, agg_ps in enumerate(agg_list):
                    kp = agg_ps.shape[0]
                    agg_sb = wpool.tile(
                        [kp, 512], dt.bfloat16, tag=f"aggsb{gi}", name=f"aggsb_{l}_{c}_{gi}"
                    )
                    nc.scalar.activation(
                        agg_sb[:], agg_ps[:], mybir.ActivationFunctionType.Copy
                    )
                    asbs.append(agg_sb)
                for gi, (agg_sb, wa) in enumerate(zip(asbs, wa_list)):
                    nc.tensor.matmul(
                        h1_ps[:],
                        wa[:],
                        agg_sb[:],
                        start=(gi == 0),
                        stop=(gi == len(asbs) - 1),
                    )
                h1_sb = wpool.tile([128, 512], dt.bfloat16, tag="h1sb", name=f"h1sb_{l}_{c}")
                nc.scalar.activation(
                    h1_sb[:],
                    h1_ps[:],
                    mybir.ActivationFunctionType.Relu,
                    bias=ba_sb[l][:, 0:1],
                )
                xn_ps = ps_mlp.tile([128, 4 * kco], dt.float32, tag="xn", name=f"xn_{l}_{c}")
                for t2 in range(4):
                    nc.tensor.matmul(
                        xn_ps[:, t2 * kco : (t2 + 1) * kco],
                        h1_sb[:, t2 * 128 : (t2 + 1) * 128],
                        wb_sb[l][:],
                        start=True,
                        stop=True,
                    )
                xe_sb = wpool.tile([128, 4 * kco], dt.float32, tag="xe", name=f"xe_{l}_{c}")
                nc.vector.tensor_add(xe_sb[:], xn_ps[:], dd4_sb[l][:])
                for t2 in range(4):
                    mcol = mask_sb[:, c * 4 + t2 : c * 4 + t2 + 1]
                    nc.scalar.activation(
                        xc_tile[:, t2 : t2 + 1, :],
                        xe_sb[:, t2 * kco : (t2 + 1) * kco],
                        mybir.ActivationFunctionType.Copy,
                        scale=mcol,
                    )

            # ================= Layer 0: fp8 DoubleRow aggregation ============
            agg0 = [
                ps_agg.tile([64, 512], dt.float32, tag=f"agg{c}", name=f"agg0_{c}")
                for c in range(4)
            ]
            for t in range(NPAIR):
                lhsT = x0_sb[:, 2 * t : 2 * t + 2, :]
                for c in range(4):
                    nc.tensor.matmul(
                        agg0[c][:],
                        lhsT,
                        adjp_sb[t][:, :, c * 512 : (c + 1) * 512],
                        start=(t == 0),
                        stop=(t == NPAIR - 1),
                        perf_mode=DR,
                    )
            xc1 = [
                xpool.tile([128, 4, KC_OUT[0]], dt.float8e4, tag=f"xc1_{c}", name=f"xc1_{c}")
                for c in range(4)
            ]
            for c in range(4):
                mlp_epilogue(0, c, None, [agg0[c]], [wa0_sb], xc1[c], last=False)
                nc.sync.dma_start(agin1[c][:, :], xc1[c][:, :, :])
                nc.gpsimd.collective_compute(
                    "AllGather",
                    mybir.AluOpType.bypass,
                    replica_groups=[[0, 1], [2, 3], [4, 5], [6, 7]],
                    ins=[agin1[c].opt()],
                    outs=[agout1[c].opt()],
                )
                for rk in range(2):
                    nc.gpsimd.dma_start(
                        xr1[rk][c][:, :, :], agout1[c][rk * 128 : (rk + 1) * 128, :]
                    )

            # ================= Layer 1: fp8 DoubleRow, two half-layers =======
            # pair order follows AllGather completion order (chunk-major)
            pair_order = [(rk, c, u) for c in range(4) for rk in range(2) for u in range(2)]
            xc2 = [
                xpool.tile([128, 4, KC_OUT[1]], dt.bfloat16, tag=f"xc2_{c}", name=f"xc2_{c}")
                for c in range(4)
            ]
            for half in range(2):
                agg1 = [
                    [
                        ps_agg.tile(
                            [64, 512], dt.float32, tag=f"agg{2 * cc + g}",
                            name=f"agg1_{half}_{cc}_{g}",
                        )
                        for g in range(2)
                    ]
                    for cc in range(2)
                ]
                for pi, (rk, c, u) in enumerate(pair_order):
                    t = rk * 8 + c * 2 + u  # adjacency pair index
                    for g in range(2):
                        lhsT = xr1[rk][c][:, 2 * u : 2 * u + 2, 64 * g : 64 * g + 64]
                        for cc in range(2):
                            c_out = 2 * half + cc
                            nc.tensor.matmul(
                                agg1[cc][g][:],
                                lhsT,
                                adjp_sb[t][:, :, c_out * 512 : (c_out + 1) * 512],
                                start=(pi == 0),
                                stop=(pi == NPAIR - 1),
                                perf_mode=DR,
                            )
                for cc in range(2):
                    c_out = 2 * half + cc
                    mlp_epilogue(
                        1, c_out, None,
                        [agg1[cc][0], agg1[cc][1]],
                        [wa1a_sb, wa1b_sb],
                        xc2[c_out],
                        last=False,
                    )
                    nc.sync.dma_start(agin2[c_out][:, :], xc2[c_out][:, :, :])
                    nc.gpsimd.collective_compute(
                        "AllGather",
                        mybir.AluOpType.bypass,
                        replica_groups=[[0, 1], [2, 3], [4, 5], [6, 7]],
                        ins=[agin2[c_out].opt()],
                        outs=[agout2[c_out].opt()],
                    )
                    for rk in range(2):
                        nc.gpsimd.dma_start(
                            xr2[rk][c_out][:, :, :],
                            agout2[c_out][rk * 128 : (rk + 1) * 128, :],
                        )

            # ================= Layer 2: bf16 x against fp8 adjacency =========
            agg2 = [
                ps_agg.tile([128, 512], dt.float32, tag=f"agg{c}", name=f"agg2_{c}")
                for c in range(4)
            ]
            tile_order = [(rk, c, u) for c in range(4) for rk in range(2) for u in range(4)]
            for ti, (rk, c, u) in enumerate(tile_order):
                t, j = divmod(c * 4 + u + rk * 16, 2)  # adjacency pair, slot
                lhsT = xr2[rk][c][:, u : u + 1, :]
                for c_out in range(4):
                    nc.tensor.matmul(
                        agg2[c_out][:],
                        lhsT,
                        adjp_sb[t][:, j : j + 1, c_out * 512 : (c_out + 1) * 512],
                        start=(ti == 0),
                        stop=(ti == len(tile_order) - 1),
                    )
            xout = [
                wpool.tile([128, 4, KC_OUT[2]], dt.float32, tag=f"xo{c}", name=f"xo_{c}")
                for c in range(4)
            ]
            for c in range(4):
                mlp_epilogue(2, c, None, [agg2[c]], [wa2_sb], xout[c], last=True)
                nc.sync.dma_start(
                    out_d[:, c * 4 * KC_OUT[2] : (c + 1) * 4 * KC_OUT[2]],
                    xout[c][:, :, :],
                )

    n_split = _legalize_sync_waits(nc)
    print(f"kernel: legalized {n_split} multi-wait instructions", file=sys.stderr)
    return nc


def get_program():
    if "nc" not in _PROGRAM_CACHE:
        _PROGRAM_CACHE["nc"] = _build_program()
    return _PROGRAM_CACHE["nc"]


def prepare_in_maps(inputs):
    """Host-side prep: fold BN into weights, transpose+slice adjacency, quantize x."""
    f32 = np.float32
    x = np.asarray(inputs["x"], f32)
    adj = np.asarray(inputs["adj"], f32)
    mask = np.asarray(inputs["mask"]).astype(bool)

    # folded per-layer constants (shared by all cores)
    const = {}
    for l in range(3):
        Wa = np.asarray(inputs[f"Wa{l}"], f32)
        ba = np.asarray(inputs[f"ba{l}"], f32)
        Wb = np.asarray(inputs[f"Wb{l}"], f32)
        bb = np.asarray(inputs[f"bb{l}"], f32)
        s1 = np.asarray(inputs[f"bng{l}"], f32) / np.sqrt(
            np.asarray(inputs[f"bnv{l}"], f32) + BN_EPS
        )
        c1 = np.asarray(inputs[f"bnb{l}"], f32) - np.asarray(inputs[f"bnm{l}"], f32) * s1
        Wb1 = s1[:, None] * Wb
        bb1 = bb + c1 @ Wb
        if l < 2:
            s2 = np.asarray(inputs[f"og{l}"], f32) / np.sqrt(
                np.asarray(inputs[f"ov{l}"], f32) + BN_EPS
            )
            c2 = np.asarray(inputs[f"ob{l}"], f32) - np.asarray(inputs[f"om{l}"], f32) * s2
            Wb2 = (Wb1 * s2[None, :]).astype(f32)
            d = (bb1 * s2 + c2).astype(f32)
        else:
            Wb2 = Wb1.astype(f32)
            d = bb1.astype(f32)
        ci, co = Wa.shape[0], Wb2.shape[1]
        waBD = np.zeros((2 * ci, 2 * H), f32)
        wbBD = np.zeros((2 * H, 2 * co), f32)
        for k in range(2):
            waBD[k * ci : (k + 1) * ci, k * H : (k + 1) * H] = Wa
            wbBD[k * H : (k + 1) * H, k * co : (k + 1) * co] = Wb2
        if l == 0:
            const["wa0"] = waBD.astype(BF16)
        elif l == 1:
            const["wa1a"] = np.ascontiguousarray(waBD[0:64, :]).astype(BF16)
            const["wa1b"] = np.ascontiguousarray(waBD[64:128, :]).astype(BF16)
        else:
            const["wa2"] = waBD.astype(BF16)
        const[f"wb{l}"] = wbBD.astype(BF16)
        const[f"ba{l}"] = np.concatenate([ba, ba]).reshape(128, 1).astype(f32)
        d2 = np.concatenate([d, d])
        const[f"dd4_{l}"] = np.broadcast_to(
            np.tile(d2, 4)[None, :], (128, 4 * 2 * co)
        ).astype(f32).copy()

    in_maps = []
    for core in range(N_CORES):
        b, half = divmod(core, 2)
        r0 = half * HALF
        # adjT[i, m] = adj[b][r0+m, i] + I -> pair-interleaved p-major layout
        adjT = np.ascontiguousarray(adj[b][r0 : r0 + HALF, :].T)
        adjT[np.arange(HALF) + r0, np.arange(HALF)] += 1.0
        adjp = (
            adjT.reshape(NPAIR, 2, 128, HALF)
            .transpose(0, 2, 1, 3)
            .reshape(NPAIR * 128, 2 * HALF)
            .astype(FP8)
        )
        xb = x[b].reshape(N, KC_IN[0]).astype(FP8)  # |x| <= ~5.1, no clip needed
        x0q = xb.reshape(32, 128, KC_IN[0]).transpose(1, 0, 2).reshape(128, 32 * KC_IN[0])
        mhalf = mask[b][r0 : r0 + HALF].astype(f32)
        m = dict(const)
        m["adjp"] = adjp
        m["x0q"] = np.ascontiguousarray(x0q)
        m["mask_cols"] = np.ascontiguousarray(mhalf.reshape(16, 128).T)
        in_maps.append(m)
    return in_maps


def gather_output(res):
    """Unshuffle per-core p-major outputs into the full (B,N,K,C_OUT) tensor."""
    out = np.zeros((B, N, K, C_OUT), np.float32)
    for core in range(N_CORES):
        b, half = divmod(core, 2)
        r0 = half * HALF
        o = res.results[core]["out"].reshape(128, 16, KC_OUT[2])
        o = o.transpose(1, 0, 2).reshape(HALF, K, C_OUT)
        out[b, r0 : r0 + HALF] = o
    return out


def run(in_maps, trace=False, **kw):
    nc = get_program()
    return run_bass_kernel_spmd(nc, in_maps, list(range(N_CORES)), trace=trace, **kw)


def kernel(**inputs) -> np.ndarray:
    in_maps = prepare_in_maps(inputs)
    res = run(in_maps)
    return gather_output(res)
